# revision 10
# baseline (speedup 1.0000x reference)
"""Trainium2 Bass kernel for HDGradientCompressionLayer forward.

Reference computation: y = einsum("bsd,df->bsf", x, W) + b
  x: (4, 4096, 1024) f32, W: (1024, 1024) f32, b: (1024,) f32.

Strategy (data-parallel across 8 cores, per sharding hint):
  Flatten x to (16384, 1024); each core gets 2048 rows (= 16 rowblocks
  of 128).  All layout work happens on the HOST so the device does pure
  HWDGE copy DMAs and the PE does only the 256 bf16 matmuls:
    - host casts x/W to bf16, pre-transposes each core's x shard to
      xT [d, m], packs W with the first 512 xT columns ("wxa") ordered
      [x_j0 | W_n0 | x_j1..3 | W_n1] so the first matmul's operands are
      one contiguous 160KB chunk, and pre-broadcasts the f32 bias (a
      plain HWDGE copy instead of a slow SWDGE replicate that would
      starve the load queue),
    - loads are split across BOTH HWDGE rings (SP + ACT) so the first
      k-tile's chunks land in parallel and the PE never waits on supply,
    - device: rowblock groups of (4,4,4,3,1) (PSUM-bank limited),
      k-outer accumulation psum[m,f] += xT[k][:,m-slice].T @ W[k][:,f];
      the 1-rowblock final group accumulates bank-major so bank n0
      evicts/stores while bank n1 is still accumulating,
    - DVE adds the bias during PSUM->SBUF eviction; the final bank is
      evicted in halves with 128KB stores alternating across both HWDGE
      rings to keep the post-matmul tail short.
"""

import os

import numpy as np

import concourse.bass as bass
import concourse.bacc as bacc
import concourse.tile as tile
from concourse import mybir
from concourse.bass_utils import run_bass_kernel_spmd

N_CORES = 8
B, S, D = 4, 4096, 1024
F = 1024
ROWS_TOTAL = B * S            # 16384
ROWS = ROWS_TOTAL // N_CORES  # 2048 per core
P = 128
NSPLIT = 512                  # one PSUM bank of f32
KB = D // P                   # 8 contraction blocks
RB = ROWS // P                # 16 rowblocks per core
GROUPS = (4, 4, 4, 3, 1)      # rowblocks per PSUM group (<=4: 8 banks)
MA = 512                      # xT columns packed with W into wxa
MB = 512                      # xT columns in wxb
MR = ROWS - MA - MB           # 1024 xT columns in xr

# wxa row layout: [ x m0:128 | W f0:512 | x m128:512 | W f512:1024 ]
XA0 = 0                       # x cols m 0:128
WN0 = P                       # W cols 0:512
XA1 = P + NSPLIT              # x cols m 128:512
WN1 = P + NSPLIT + (MA - P)   # W cols 512:1024
WXA_COLS = F + MA
K0SPLIT = XA1                 # k0 chunk1 = [0:640] (x_j0 + W_n0)

_BF16 = mybir.dt.np(mybir.dt.bfloat16)


def build_nc() -> bass.Bass:
    nc = bacc.Bacc("TRN2", target_bir_lowering=False, debug=False)
    wxa = nc.dram_tensor("wxa", [D, WXA_COLS], mybir.dt.bfloat16, kind="ExternalInput").ap()
    wxb = nc.dram_tensor("wxb", [D, MB], mybir.dt.bfloat16, kind="ExternalInput").ap()
    xr = nc.dram_tensor("xr", [D, MR], mybir.dt.bfloat16, kind="ExternalInput").ap()
    bb = nc.dram_tensor("bb", [P, F], mybir.dt.float32, kind="ExternalInput").ap()
    y = nc.dram_tensor("y", [ROWS, F], mybir.dt.float32, kind="ExternalOutput").ap()

    with tile.TileContext(nc) as tc:
        with tc.tile_pool(name="const", bufs=1) as const, \
             tc.tile_pool(name="ap", bufs=1) as apool, \
             tc.tile_pool(name="bp", bufs=1) as bpool, \
             tc.tile_pool(name="rp", bufs=1) as rpool, \
             tc.tile_pool(name="yp", bufs=1) as yp, \
             tc.tile_pool(name="psp", bufs=1, space="PSUM") as psp:

            # HAM warmup: full-array matmuls on a zeroed tile (1-partition
            # ones do NOT register as PE activity) bridge the window from
            # the preamble to the first data arrival; 256-col dummies give
            # finer handoff granularity than 512-col ones.
            warm = const.tile([P, NSPLIT], mybir.dt.bfloat16)
            nc.vector.memset(warm[:], 0.0)
            warm_ps = psp.tile([P, NSPLIT], mybir.dt.float32, tag="ps_0_0", bufs=1)
            for _ in range(10):
                nc.tensor.matmul(warm_ps[:, 0:NSPLIT // 2], warm[:, 0:P],
                                 warm[:, 0:NSPLIT // 2],
                                 start=True, stop=True, skip_group_check=True)

            # Loads split across both HWDGE rings (SP=sync, ACT=scalar) in
            # consumption order; the k0 tile is split so the first matmul's
            # 160KB chunk and the rest arrive in parallel on the two rings.
            wxa_t = [apool.tile([P, WXA_COLS], mybir.dt.bfloat16, name=f"wxa{k}", tag=f"wxa{k}")
                     for k in range(KB)]
            wxb_t = [bpool.tile([P, MB], mybir.dt.bfloat16, name=f"wxb{k}", tag=f"wxb{k}")
                     for k in range(KB)]
            xr_t = [rpool.tile([P, MR], mybir.dt.bfloat16, name=f"xr{k}", tag=f"xr{k}")
                    for k in range(KB)]
            b_bc = const.tile([P, F], mybir.dt.float32)

            # SP ring: k0 first chunk (x_j0 + W_n0), even k tiles, then xr.
            nc.sync.dma_start(wxa_t[0][:, :K0SPLIT], wxa[0:P, :K0SPLIT])
            for k in (2, 4, 6):
                nc.sync.dma_start(wxa_t[k][:], wxa[k * P:(k + 1) * P, :])
            for k in range(KB):
                nc.sync.dma_start(xr_t[k][:], xr[k * P:(k + 1) * P, :])
            # ACT ring: k0 second chunk, odd k tiles, bias, then wxb.
            nc.scalar.dma_start(wxa_t[0][:, K0SPLIT:], wxa[0:P, K0SPLIT:])
            for k in (1, 3, 5, 7):
                nc.scalar.dma_start(wxa_t[k][:], wxa[k * P:(k + 1) * P, :])
            nc.scalar.dma_start(b_bc[:], bb[:, :])
            for k in range(KB):
                nc.scalar.dma_start(wxb_t[k][:], wxb[k * P:(k + 1) * P, :])

            def xslice(k: int, rb: int):
                m0 = rb * P
                if m0 == 0:
                    return wxa_t[k][:, XA0:XA0 + P]
                if m0 < MA:
                    return wxa_t[k][:, XA1 + m0 - P:XA1 + m0]
                if m0 < MA + MB:
                    return wxb_t[k][:, m0 - MA:m0 - MA + P]
                return xr_t[k][:, m0 - MA - MB:m0 - MA - MB + P]

            def wslice(k: int, n: int):
                base = WN0 if n == 0 else WN1
                return wxa_t[k][:, base:base + NSPLIT]

            rb0 = 0
            for gi, gsz in enumerate(GROUPS):
                # The 1-rowblock final group takes the ps_3 tags (last
                # evicted two groups ago) so its k=0 matmul never waits.
                joff = 3 if gsz == 1 else 0
                ps = [[psp.tile([P, NSPLIT], mybir.dt.float32, name=f"ps_{j + joff}_{n}",
                                tag=f"ps_{j + joff}_{n}", bufs=1)
                       for n in range(2)] for j in range(gsz)]
                if gsz == 1:
                    # Final rowblock: bank-major accumulation so bank n0 is
                    # complete (and evicting/storing) while n1 accumulates;
                    # after the last matmul only one bank remains to drain.
                    rb = rb0
                    y_sb = yp.tile([P, F], mybir.dt.float32, name="ysb", tag="ysb", bufs=6)
                    for n in range(2):
                        for k in range(KB):
                            nc.tensor.matmul(
                                ps[0][n][:], xslice(k, rb), wslice(k, n),
                                start=(k == 0), stop=(k == KB - 1),
                            )
                        for h in range(2):
                            c0 = n * NSPLIT + h * (NSPLIT // 2)
                            c1 = c0 + NSPLIT // 2
                            nc.vector.tensor_add(
                                y_sb[:, c0:c1],
                                ps[0][n][:, h * (NSPLIT // 2):(h + 1) * (NSPLIT // 2)],
                                b_bc[:, c0:c1],
                            )
                            eng = nc.scalar if h == 0 else nc.sync
                            eng.dma_start(y[rb * P:(rb + 1) * P, c0:c1], y_sb[:, c0:c1])
                    rb0 += gsz
                    continue
                for k in range(KB):
                    for j in range(gsz):
                        xs = xslice(k, rb0 + j)
                        for n in range(2):
                            nc.tensor.matmul(
                                ps[j][n][:], xs, wslice(k, n),
                                start=(k == 0), stop=(k == KB - 1),
                            )
                for j in range(gsz):
                    rb = rb0 + j
                    y_sb = yp.tile([P, F], mybir.dt.float32, name="ysb", tag="ysb", bufs=6)
                    for n in range(2):
                        nc.vector.tensor_add(
                            y_sb[:, n * NSPLIT:(n + 1) * NSPLIT],
                            ps[j][n][:],
                            b_bc[:, n * NSPLIT:(n + 1) * NSPLIT],
                        )
                    nc.scalar.dma_start(y[rb * P:(rb + 1) * P, :], y_sb[:])
                rb0 += gsz

    nc.compile()
    return nc


_NC_CACHE: dict[str, bass.Bass] = {}


def _get_nc() -> bass.Bass:
    if "nc" not in _NC_CACHE:
        _NC_CACHE["nc"] = build_nc()
    return _NC_CACHE["nc"]


def _run(in_maps, trace: bool = False):
    nc = _get_nc()
    return run_bass_kernel_spmd(nc, in_maps, list(range(N_CORES)), trace=trace)


def make_in_maps(x: np.ndarray, W: np.ndarray, b: np.ndarray):
    x = np.ascontiguousarray(np.asarray(x, dtype=np.float32)).reshape(ROWS_TOTAL, D)
    W_bf = np.asarray(W, dtype=np.float32).astype(_BF16)
    b = np.asarray(b, dtype=np.float32)
    bb = np.ascontiguousarray(np.broadcast_to(b[None, :], (P, F)))
    in_maps = []
    for c in range(N_CORES):
        shard_bf = x[c * ROWS:(c + 1) * ROWS].astype(_BF16)  # [2048, 1024]
        wxa = np.empty((D, WXA_COLS), dtype=_BF16)
        wxa[:, XA0:XA0 + P] = shard_bf[0:P].T
        wxa[:, WN0:WN0 + NSPLIT] = W_bf[:, 0:NSPLIT]
        wxa[:, XA1:XA1 + (MA - P)] = shard_bf[P:MA].T
        wxa[:, WN1:WN1 + NSPLIT] = W_bf[:, NSPLIT:F]
        wxb = np.ascontiguousarray(shard_bf[MA:MA + MB].T)
        xr = np.ascontiguousarray(shard_bf[MA + MB:].T)
        in_maps.append({"wxa": wxa, "wxb": wxb, "xr": xr, "bb": bb})
    return in_maps


def kernel(x: np.ndarray, W: np.ndarray, b: np.ndarray) -> np.ndarray:
    in_maps = make_in_maps(x, W, b)
    res = _run(in_maps, trace=bool(int(os.environ.get("BASS_KERNEL_TRACE", "0"))))
    y = np.concatenate([res.results[c]["y"] for c in range(N_CORES)], axis=0)
    return y.reshape(B, S, F)
